# revision 1
# baseline (speedup 1.0000x reference)
"""Trainium2 Bass kernel for a 40-layer planar-flow chain (nn_Encoder_27676769255710).

Reference computation (per layer l, sequential over 40 layers):
    u_hat_l = u_l + ((-1 + softplus(w_l.u_l)) - w_l.u_l) * w_l / (w_l.w_l)
    act_l   = tanh(X_l @ w_l + b_l)
    X_{l+1} = X_l + act_l[:, None] * u_hat_l

Algebraic reformulation (u_hat and C depend only on params -> host precompute):
    C[m, j]  = w_m . u_hat_j                       (40x40)
    Y0       = X_0 @ W^T + b                       (one big matmul)
    P        = Y0;  for l: act_l = tanh(P[:, l]);  P[:, l+1:] += act_l * C[l+1:, l]
    X_out    = X_0 + A @ U_hat                     (one big matmul)

So X is read once and written once (memory-roofline), the big work is two
PE matmuls in bf16, and the sequential part is a tiny 40-step recurrence
on [S, 40].

The X@W^T contraction runs over the feature dim, which must sit on SBUF
partitions: X is cast to bf16 and transposed on the PE in 128x128 chunks
(bf16 halves both the weight-load and the stream cost vs fp32 LOW_HIGH
mode), with the PSUM->SBUF copies split between the Scalar and Vector
engines per row-block so neither becomes the pipeline pacer and the tanh
recurrence never thrashes the ACT activation table.

Sharding: data-parallel on the batch axis, 2048 rows -> 8 cores x 256 rows.
Params replicated.
"""

import os
import sys
from contextlib import ExitStack

import numpy as np

for _p in ("/opt/trn_rl_repo",):
    if os.path.isdir(_p) and _p not in sys.path:
        sys.path.append(_p)

import ml_dtypes

import concourse.bacc as bacc
import concourse.bass as bass
import concourse.mybir as mybir
import concourse.tile as tile
from concourse.bass_utils import run_bass_kernel_spmd

BF16 = ml_dtypes.bfloat16

S, D, L = 2048, 16384, 40
NCORES = 8
SS = S // NCORES          # 256 rows per core
NB = SS // 128            # 2 row-blocks of 128 per core
NCHUNK = D // 128         # 128 d-chunks for the transposed X@W^T contraction
XDMA = 8                  # 1MB input DMAs
XOUT = 8                  # out-DMA groups per block
XCAST = 8                 # cast granularity within phase 1

f32 = mybir.dt.float32
bf16 = mybir.dt.bfloat16

_CACHE = {}


def _build_nc():
    nc = bacc.Bacc(
        "TRN2",
        target_bir_lowering=False,
        debug=False,
        num_devices=NCORES,
    )

    x_d = nc.dram_tensor("x", [SS, D], f32, kind="ExternalInput").ap()
    wt_d = nc.dram_tensor("wt", [128, NCHUNK * L], bf16, kind="ExternalInput").ap()
    uh_d = nc.dram_tensor("uh", [L, D], bf16, kind="ExternalInput").ap()
    ct_d = nc.dram_tensor("ct", [128, L * L], f32, kind="ExternalInput").ap()
    br_d = nc.dram_tensor("br", [128, L], f32, kind="ExternalInput").ap()
    id16_d = nc.dram_tensor("id16", [128, 128], bf16, kind="ExternalInput").ap()
    y_d = nc.dram_tensor("y", [SS, D], f32, kind="ExternalOutput").ap()

    with tile.TileContext(nc) as tc, ExitStack() as ctx:
        sb = ctx.enter_context(tc.tile_pool(name="sb", bufs=1))
        xbfp = ctx.enter_context(tc.tile_pool(name="xbfp", bufs=2))
        xtp = ctx.enter_context(tc.tile_pool(name="xtp", bufs=3))
        prp = ctx.enter_context(tc.tile_pool(name="prp", bufs=NB))
        psT = ctx.enter_context(
            tc.tile_pool(name="psT", bufs=2, space=bass.MemorySpace.PSUM)
        )
        psY = ctx.enter_context(
            tc.tile_pool(name="psY", bufs=2, space=bass.MemorySpace.PSUM)
        )
        psU = ctx.enter_context(
            tc.tile_pool(name="psU", bufs=2, space=bass.MemorySpace.PSUM)
        )

        # --- resident tensors ---
        x_sb = sb.tile([128, NB, D], f32)          # whole X shard, updated in place
        wt_sb = sb.tile([128, NCHUNK * L], bf16)   # W^T chunk-packed
        uh_sb = sb.tile([L, D], bf16)              # u_hat
        ct_sb = sb.tile([128, L * L], f32)         # C^T replicated per partition
        br_sb = sb.tile([128, L], f32)             # b replicated
        id16 = sb.tile([128, 128], bf16)

        lw = D // XDMA
        xw = D // XCAST
        # input stream: all on the sync ring (per-queue BW is best unshared)
        for b in range(NB):
            for g in range(XDMA):
                nc.sync.dma_start(
                    x_sb[:, b, g * lw : (g + 1) * lw],
                    x_d[b * 128 : (b + 1) * 128, g * lw : (g + 1) * lw],
                )
        # params go on the scalar HWDGE ring so they don't queue behind X
        nc.scalar.dma_start(id16[:], id16_d[:])
        nc.scalar.dma_start(wt_sb[:], wt_d[:])
        nc.scalar.dma_start(ct_sb[:], ct_d[:])
        nc.scalar.dma_start(br_sb[:], br_d[:])
        nc.scalar.dma_start(uh_sb[:], uh_d[:])

        # --- phase 1 (per block): cast X->bf16, PE-transpose chunks (bf16),
        #     copy PSUM->SBUF, accumulate Y0[b] = X_b @ W^T.
        #     Block 0 uses ACT for cast+copy, block 1 uses DVE, so block 0's
        #     tanh recurrence can overlap block 1's phase 1 without ACT
        #     activation-table thrash. ---
        y0_ps = []
        for _b in range(NB):
            y0_b = psY.tile([128, L], f32, tag="y0", name=f"y0_{_b}")
            y0_ps.append(y0_b)
        CG = 4

        def phase1(b):
            for g in range(XCAST):
                xbf = xbfp.tile([128, xw], bf16, tag="xbf", name=f"xbf_{b}_{g}")
                if b == 0:
                    nc.scalar.copy(xbf[:], x_sb[:, b, g * xw : (g + 1) * xw])
                else:
                    nc.vector.tensor_copy(xbf[:], x_sb[:, b, g * xw : (g + 1) * xw])
                for cg in range(xw // (CG * 128)):
                    t_ps = psT.tile(
                        [128, CG * 128], bf16, tag="tps", name=f"tps_{b}_{g}_{cg}"
                    )
                    for i in range(CG):
                        r = cg * CG + i
                        nc.tensor.transpose(
                            t_ps[:, i * 128 : (i + 1) * 128],
                            xbf[:, r * 128 : (r + 1) * 128],
                            id16[:],
                        )
                    xt = xtp.tile(
                        [128, CG * 128], bf16, tag="xt", name=f"xt_{b}_{g}_{cg}"
                    )
                    if b == 0:
                        nc.scalar.copy(xt[:], t_ps[:])
                    else:
                        nc.vector.tensor_copy(xt[:], t_ps[:])
                    for i in range(CG):
                        c = g * (xw // 128) + cg * CG + i
                        nc.tensor.matmul(
                            y0_ps[b][:],
                            xt[:, i * 128 : (i + 1) * 128],
                            wt_sb[:, c * L : (c + 1) * L],
                            start=(c == 0),
                            stop=(c == NCHUNK - 1),
                        )

        def recurrence(b):
            p_t = prp.tile([128, L], f32, tag="p", name=f"p_{b}")
            a_t = prp.tile([128, L], bf16, tag="a", name=f"a_{b}")
            nc.vector.tensor_add(p_t[:], y0_ps[b][:], br_sb[:])
            for l in range(L):
                nc.scalar.activation(
                    a_t[:, l : l + 1],
                    p_t[:, l : l + 1],
                    mybir.ActivationFunctionType.Tanh,
                )
                if l + 1 < L:
                    nc.vector.scalar_tensor_tensor(
                        out=p_t[:, l + 1 :],
                        in0=ct_sb[:, l * L + l + 1 : l * L + L],
                        scalar=a_t[:, l : l + 1],
                        in1=p_t[:, l + 1 :],
                        op0=mybir.AluOpType.mult,
                        op1=mybir.AluOpType.add,
                    )
            # A -> A^T for the update matmul
            at_ps = psY.tile([L, 128], bf16, tag="y0", name=f"at_ps_{b}")
            nc.tensor.transpose(at_ps[:], a_t[:], id16[:])
            at_t = prp.tile([L, 128], bf16, tag="at", name=f"at_{b}")
            nc.vector.tensor_copy(at_t[:], at_ps[:])
            return at_t

        def update(b, at_t):
            # X_out = X + A @ U_hat, streamed out
            ow = D // XOUT
            for n in range(D // 1024):
                u_ps = psU.tile([128, 1024], f32, tag="ups", name=f"ups_{b}_{n}")
                for h in range(2):
                    nc.tensor.matmul(
                        u_ps[:, h * 512 : (h + 1) * 512],
                        at_t[:],
                        uh_sb[:, n * 1024 + h * 512 : n * 1024 + (h + 1) * 512],
                        start=True,
                        stop=True,
                    )
                nc.vector.tensor_add(
                    x_sb[:, b, n * 1024 : (n + 1) * 1024],
                    u_ps[:],
                    x_sb[:, b, n * 1024 : (n + 1) * 1024],
                )
                if (n + 1) % (ow // 1024) == 0:
                    g = n // (ow // 1024)
                    deng = nc.sync if b == 0 else nc.scalar
                    deng.dma_start(
                        y_d[b * 128 : (b + 1) * 128, g * ow : (g + 1) * ow],
                        x_sb[:, b, g * ow : (g + 1) * ow],
                    )

        phase1(0)
        phase1(1)
        at0 = recurrence(0)
        update(0, at0)
        at1 = recurrence(1)
        update(1, at1)

    nc.compile()
    return nc


def _prep_params(ws: np.ndarray, us: np.ndarray, bs: np.ndarray) -> dict:
    """Host-side precompute of the tiny flow-parameter tensors (f64 for accuracy)."""
    w = ws.astype(np.float64)
    u = us.astype(np.float64)
    wu = np.sum(w * u, axis=1)
    ww = np.sum(w * w, axis=1)
    m = -1.0 + np.logaddexp(0.0, wu)  # softplus
    u_hat = u + ((m - wu) / ww)[:, None] * w              # [L, D]
    C = w @ u_hat.T                                        # C[m, j] = w_m . u_hat_j

    # W^T packed for the chunked contraction: wt[p, c*L + l] = W[l, c*128 + p]
    wt = np.ascontiguousarray(
        ws.astype(np.float32).T.reshape(NCHUNK, 128, L).transpose(1, 0, 2)
    ).reshape(128, NCHUNK * L)

    # C^T replicated per partition: ct[p, j*L + m] = C[m, j]
    ct = np.tile(np.ascontiguousarray(C.T.astype(np.float32)).reshape(1, L * L), (128, 1))
    br = np.tile(bs.astype(np.float32).reshape(1, L), (128, 1))

    return {
        "wt": wt.astype(BF16),
        "uh": u_hat.astype(np.float32).astype(BF16),
        "ct": np.ascontiguousarray(ct, dtype=np.float32),
        "br": np.ascontiguousarray(br, dtype=np.float32),
        "id16": np.eye(128, dtype=np.float32).astype(BF16),
    }


def run(X, ws, us, bs, trace=False, **trace_kwargs):
    if "nc" not in _CACHE:
        _CACHE["nc"] = _build_nc()
    nc = _CACHE["nc"]

    params = _prep_params(np.asarray(ws), np.asarray(us), np.asarray(bs))
    X = np.ascontiguousarray(np.asarray(X, dtype=np.float32))
    in_maps = [
        {"x": X[c * SS : (c + 1) * SS], **params} for c in range(NCORES)
    ]
    res = run_bass_kernel_spmd(
        nc, in_maps, list(range(NCORES)), trace=trace, **trace_kwargs
    )
    out = np.concatenate([res.results[c]["y"] for c in range(NCORES)], axis=0)
    return out, res


def kernel(X, ws, us, bs):
    out, _ = run(X, ws, us, bs, trace=False)
    return out

